# revision 1
# baseline (speedup 1.0000x reference)
"""Trainium2 Bass kernel for GQA attention (B=2, T=2048, C=4096, H=32, KV=8, D=128)
with RoPE and causal mask.

Sharding: tensor-parallel over heads across 8 cores. Each core owns 4 Q heads and
their shared KV head: projects q/k/v for those heads, runs causal attention, and
computes a partial output projection; the host sums the 8 partials.

All on-chip layouts are transposed ([feature, token]) so every matmul consumes
natural slices:
  qT/kT/vT = W^T @ x  via lhsT=W-tile [128c, cols], rhs=xT-tile [128c, 512t]
  sT[tk, tq] = kT-tile^T @ qT-chunk   (per 128-row key tile x 512-col query chunk)
  pT = exp(sT/sqrt(D) - 10) on ACT; strictly-causal-upper tiles skipped entirely
  yT[d, tq] += v-tile^T @ pT          (v pre-transposed to [t, d] via PE transpose)
  out[tq, :] += yT_h^T @ wo_h         (accumulate 4 heads in PSUM, evict, DMA out)
Softmax denominator: a ones-matrix matmul accumulates sum(pT) into a [128,512]
PSUM broadcast alongside the attn@v accumulation; reciprocal_approx_fast +
one DVE multiply normalize yT. Output-projection matmul "jobs" are popped from
a queue inside the score streams to keep the in-order PE queue dense while ACT
works through the exps.
"""

import os
from collections import deque
from contextlib import ExitStack

import numpy as np
import ml_dtypes

import concourse.bacc as bacc
import concourse.mybir as mybir
import concourse.tile as tile

BF = mybir.dt.bfloat16
F32 = mybir.dt.float32
AFT = mybir.ActivationFunctionType

NCORES = 8
B, T, C = 2, 2048, 4096
H, KV, D = 32, 8, 128
QH = H // NCORES          # 4 q-heads per core
CT = C // 128             # 32 contraction tiles
NCH = T // 512            # 4 query chunks per batch
SCALE = 1.0 / float(np.sqrt(D))
EXP_BIAS = -10.0
ROPE_BASE = 10000.0

bf16 = ml_dtypes.bfloat16


def emit_program():
    nc = bacc.Bacc("TRN2", target_bir_lowering=False, debug=False,
                   num_devices=NCORES)

    xT_d = nc.dram_tensor("xT", [C, B * T], BF, kind="ExternalInput").ap()
    wq_d = nc.dram_tensor("wq", [C, QH * D], BF, kind="ExternalInput").ap()
    wk_d = nc.dram_tensor("wk", [C, D], BF, kind="ExternalInput").ap()
    wv_d = nc.dram_tensor("wv", [C, D], BF, kind="ExternalInput").ap()
    wo_d = nc.dram_tensor("woA", [128, QH, C], BF, kind="ExternalInput").ap()
    cos_d = nc.dram_tensor("cosT", [D, T], BF, kind="ExternalInput").ap()
    sin_d = nc.dram_tensor("sinTr", [D, T], BF, kind="ExternalInput").ap()
    alw_d = nc.dram_tensor("allowA", [128, 4, 512], BF, kind="ExternalInput").ap()
    id_d = nc.dram_tensor("ident", [128, 128], BF, kind="ExternalInput").ap()
    out_d = nc.dram_tensor("out", [B * T, C], F32, kind="ExternalOutput").ap()

    with tile.TileContext(nc) as tc, ExitStack() as ctx:
        const = ctx.enter_context(tc.tile_pool(name="const", bufs=1))
        act = ctx.enter_context(tc.tile_pool(name="act", bufs=1))
        work = ctx.enter_context(tc.tile_pool(name="work", bufs=1))

        # weights + tables on the gpsimd DMA queue so they never sit ahead of
        # the xt activation loads (sync queue); chunked in 8-c-tile groups so
        # the first projection matmuls wait on ~1MB, not the full tensors
        wq_sb = const.tile([128, CT, QH * D], BF)
        wk_sb = const.tile([128, CT, D], BF)
        wv_sb = const.tile([128, CT, D], BF)
        wqr = wq_d.rearrange("(ci p) n -> p ci n", p=128)
        wkr = wk_d.rearrange("(ci p) n -> p ci n", p=128)
        wvr = wv_d.rearrange("(ci p) n -> p ci n", p=128)
        GW = 8
        for g in range(0, CT, GW):
            s = slice(g, g + GW)
            nc.gpsimd.dma_start(wq_sb[:, s, :], wqr[:, s, :])
            nc.gpsimd.dma_start(wk_sb[:, s, :], wkr[:, s, :])
            nc.gpsimd.dma_start(wv_sb[:, s, :], wvr[:, s, :])
        cos_sb = const.tile([D, T], BF)
        nc.gpsimd.dma_start(cos_sb[:], cos_d)
        sin_sb = const.tile([D, T], BF)
        nc.gpsimd.dma_start(sin_sb[:], sin_d)
        alw_sb = const.tile([128, 4, 512], BF)
        nc.gpsimd.dma_start(alw_sb[:], alw_d)
        id_sb = const.tile([128, 128], BF)
        nc.gpsimd.dma_start(id_sb[:], id_d)
        wo_sb = const.tile([128, QH, C], BF)
        nc.gpsimd.dma_start(wo_sb[:], wo_d)
        onesbf_sb = const.tile([128, 128], BF)
        nc.gpsimd.memset(onesbf_sb[:], 1.0)
        bias_sb = const.tile([128, 1], F32)
        nc.gpsimd.memset(bias_sb[:], EXP_BIAS)

        def rope_evict(dst, psum, cs):
            # dst = psum * cos + swap_halves(psum) * sin_rot   (bf16 out)
            sw = work.tile([128, 512], F32, tag="sw", bufs=3, name="sw")
            nc.vector.tensor_copy(sw[0:64, :], psum[64:128, :])
            nc.vector.tensor_copy(sw[64:128, :], psum[0:64, :])
            nc.vector.tensor_mul(sw[:], sw[:], sin_sb[:, cs])
            cst = work.tile([128, 512], F32, tag="cst", bufs=3, name="cst")
            nc.vector.tensor_mul(cst[:], psum[:], cos_sb[:, cs])
            nc.vector.tensor_add(dst, cst[:], sw[:])

        for b in range(B):
            qT = act.tile([D, QH, T], BF, tag="qT", name="qT")
            kT = act.tile([D, T], BF, tag="kT", name="kT")
            vT = act.tile([D, T], BF, tag="vT", name="vT")
            vsb = act.tile([128, T // 128, D], BF, tag="v", name="vsb")

            # ---- projections ----
            with tc.tile_pool(name=f"pproj{b}", bufs=1, space="PSUM") as pp:
                for jc in range(NCH):
                    pq = [pp.tile([128, 512], F32, tag=f"pq{h}", name=f"pq{h}")
                          for h in range(QH)]
                    pk = pp.tile([128, 512], F32, tag="pk", name="pk")
                    pv = pp.tile([128, 512], F32, tag="pv", name="pv")
                    # q matmuls run SKEW c-tiles behind k/v so the previous
                    # chunk's pq bank evictions are hidden behind ready work
                    SKEW = 4
                    xts = {}
                    col0 = b * T + 512 * jc

                    def q_mms(cq):
                        for h in range(QH):
                            nc.tensor.matmul(
                                pq[h][:], wq_sb[:, cq, 128 * h:128 * (h + 1)],
                                xts[cq][:], start=cq == 0, stop=cq == CT - 1)
                        if cq >= SKEW:
                            del xts[cq - SKEW]

                    for ci in range(CT):
                        xt = work.tile([128, 512], BF, tag="xt", bufs=10, name="xt")
                        xts[ci] = xt
                        nc.sync.dma_start(
                            xt[:], xT_d[128 * ci:128 * (ci + 1), col0:col0 + 512])
                        st, sp = ci == 0, ci == CT - 1
                        nc.tensor.matmul(pk[:], wk_sb[:, ci, :], xt[:],
                                         start=st, stop=sp)
                        nc.tensor.matmul(pv[:], wv_sb[:, ci, :], xt[:],
                                         start=st, stop=sp)
                        if ci >= SKEW:
                            q_mms(ci - SKEW)
                    for cq in range(CT - SKEW, CT):
                        q_mms(cq)
                    cs = slice(512 * jc, 512 * (jc + 1))
                    nc.scalar.copy(vT[:, cs], pv[:])
                    rope_evict(kT[:, cs], pk[:], cs)
                    for h in range(QH):
                        rope_evict(qT[:, h, cs], pq[h][:], cs)

            # ---- transpose v to [t, d] tiles ----
            with tc.tile_pool(name=f"ptr{b}", bufs=2, space="PSUM") as ptr:
                for k in range(T // 128):
                    tp = ptr.tile([128, 128], BF, tag="tp", name="tp")
                    nc.tensor.transpose(tp[:], vT[:, 128 * k:128 * (k + 1)],
                                        id_sb[:])
                    nc.vector.tensor_copy(vsb[:, k, :], tp[:])

            # ---- attention + output projection ----
            with tc.tile_pool(name=f"pattn{b}", bufs=1, space="PSUM") as pa:
                wo_jobs = deque()

                def make_wo_job(b, j, tl, o, yts):
                    def job():
                        ops = pa.tile([128, 512], F32, tag="ops", bufs=2,
                                      name="ops")
                        for h in range(QH):
                            nc.tensor.matmul(
                                ops[:], yts[h][:, 128 * tl:128 * (tl + 1)],
                                wo_sb[:, h, 512 * o:512 * (o + 1)],
                                start=h == 0, stop=h == QH - 1)
                        ob = work.tile([128, 512], F32, tag="ob", bufs=4,
                                       name="ob")
                        nc.vector.tensor_copy(ob[:], ops[:])
                        r0 = b * T + 512 * j + 128 * tl
                        nc.sync.dma_start(out_d[r0:r0 + 128, 512 * o:512 * (o + 1)],
                                          ob[:])
                    return job

                for j in range(NCH):
                    yts = {}
                    for h in range(QH):
                        yps = pa.tile([128, 512], F32, tag="yps", bufs=1,
                                      name="yps")
                        dps = pa.tile([128, 512], F32, tag="dps", bufs=1,
                                      name="dps")
                        K = 4 * j + 4
                        # pass 1: score matmuls stream; exp/mask trail on
                        # ACT/DVE (sps slots ping-pong at exp pace)
                        pts = []
                        for k in range(K):
                            sps = pa.tile([128, 512], F32, tag="sps", bufs=4,
                                          name="sps")
                            nc.tensor.matmul(
                                sps[:], kT[:, 128 * k:128 * (k + 1)],
                                qT[:, h, 512 * j:512 * (j + 1)],
                                start=True, stop=True)
                            pt = work.tile([128, 512], BF, tag="pt", bufs=18,
                                           name="pt")
                            nc.scalar.activation(pt[:], sps[:], AFT.Exp,
                                                 bias=bias_sb[:], scale=SCALE)
                            o = k - 4 * j
                            if o >= 0:
                                nc.vector.tensor_mul(pt[:], pt[:],
                                                     alw_sb[:, o, :])
                            pts.append(pt)
                            if wo_jobs:
                                wo_jobs.popleft()()
                        # pass 2: denominator + attn@v accumulation (dense
                        # PE, no wo pops: keeps DVE clear so the reciprocal
                        # lands right behind the denominator stop)
                        for k in range(K):
                            nc.tensor.matmul(dps[:], onesbf_sb[:], pts[k][:],
                                             start=(k == 0), stop=(k == K - 1))
                            nc.tensor.matmul(yps[:], vsb[:, k, :], pts[k][:],
                                             start=(k == 0), stop=(k == K - 1))
                        rec = work.tile([128, 512], F32, tag="rec", bufs=2,
                                        name="rec")
                        nc.vector.reciprocal_approx_fast(rec[:], dps[:])
                        yt = work.tile([128, 512], BF, tag="yt", bufs=8,
                                       name="yt")
                        nc.vector.tensor_mul(yt[:], yps[:], rec[:])
                        yts[h] = yt
                    for tl in range(4):
                        for o in range(C // 512):
                            wo_jobs.append(make_wo_job(b, j, tl, o, yts))
                while wo_jobs:
                    wo_jobs.popleft()()

    nc.compile()
    return nc


def host_prep(inputs):
    x = np.asarray(inputs["x"], np.float32)
    mask = np.asarray(inputs["mask"], np.float32)
    wq = np.asarray(inputs["wq"], np.float32)
    wk = np.asarray(inputs["wk"], np.float32)
    wv = np.asarray(inputs["wv"], np.float32)
    wo = np.asarray(inputs["wo"], np.float32)

    xT = np.ascontiguousarray(x.reshape(B * T, C).T).astype(bf16)
    inv = 1.0 / (ROPE_BASE ** (np.arange(0, D, 2, dtype=np.float64) / D))
    freqs = np.arange(T, dtype=np.float64)[:, None] * inv[None, :] * B
    emb = np.concatenate([freqs, freqs], axis=-1)       # [T, D]
    cosT = np.cos(emb).T.astype(np.float32).astype(bf16)
    sinT = np.sin(emb).T.astype(np.float32)
    sinT[: D // 2] *= -1.0
    sinTr = sinT.astype(bf16)
    # allow[p, o, jj] = 1 - mask[jj, 128*o + p]  (from the actual mask input)
    allowA = np.ascontiguousarray(
        np.stack([(1.0 - mask[0:512, 128 * o:128 * (o + 1)]).T
                  for o in range(4)], axis=1)).astype(bf16)   # [128, 4, 512]
    ident = np.eye(128, dtype=np.float32).astype(bf16)

    common = dict(xT=xT, cosT=cosT, sinTr=sinTr, allowA=allowA, ident=ident)
    in_maps = []
    for c in range(NCORES):
        m = dict(common)
        m["wq"] = np.ascontiguousarray(wq[:, 512 * c:512 * (c + 1)]).astype(bf16)
        m["wk"] = np.ascontiguousarray(wk[:, 128 * c:128 * (c + 1)]).astype(bf16)
        m["wv"] = np.ascontiguousarray(wv[:, 128 * c:128 * (c + 1)]).astype(bf16)
        m["woA"] = np.ascontiguousarray(
            wo[512 * c:512 * (c + 1), :].reshape(QH, 128, C)
            .transpose(1, 0, 2)).astype(bf16)
        in_maps.append(m)
    return in_maps


def kernel(**inputs) -> np.ndarray:
    from concourse.bass_utils import run_bass_kernel_spmd

    in_maps = host_prep(inputs)
    nc = emit_program()
    trace = bool(os.environ.get("BASS_KERNEL_TRACE"))
    res = run_bass_kernel_spmd(nc, in_maps, core_ids=list(range(NCORES)),
                               trace=trace)
    if trace and res.exec_time_ns is not None:
        print(f"HW exec time: {res.exec_time_ns} ns")
        if res.instructions_and_trace is not None:
            print("trace:", res.instructions_and_trace[1])
    total = np.zeros((B * T, C), np.float32)
    for r in res.results:
        total += r["out"]
    return total.reshape(B, T, C)



# revision 2
# speedup vs baseline: 1.0316x; 1.0316x over previous
"""Trainium2 Bass kernel for GQA attention (B=2, T=2048, C=4096, H=32, KV=8, D=128)
with RoPE and causal mask.

Sharding: tensor-parallel over heads across 8 cores. Each core owns 4 Q heads and
their shared KV head: projects q/k/v for those heads, runs causal attention, and
computes a partial output projection; the host sums the 8 partials (bf16 partials,
f32 accumulation on host).

All on-chip layouts are transposed ([feature, token]) so every matmul consumes
natural slices:
  qT/kT/vT = W^T @ x  via lhsT=W-tile [128c, cols], rhs=xT-tile [128c, 512t]
  sT[tk, tq] = kT-tile^T @ qT-chunk   (per 128-row key tile x 512-col query chunk;
               diagonal tiles stream only their unmasked column range)
  pT = exp(sT/sqrt(D) - 10) on ACT; strictly-causal-upper tiles skipped entirely
  S  = sum_k pT  accumulated on DVE (bf16) -> one ones-matmul per (b,h,j) gives
       the softmax denominator broadcast in PSUM (replaces a ones-matmul per tile)
  yT[d, tq] += v-tile^T @ pT          (v laid out [t, d] via DMA-crossbar transpose)
  out[tq, :] += yT_h^T @ wo_h         (accumulate 4 heads in PSUM, evict bf16, DMA)

Phase order is P(b0) P(b1) A(b0) A(b1) with double-buffered qT/kT/vsb so the PE
never sees a projection<->attention boundary stall. Output-projection matmul
"jobs" are popped from a queue inside the attention streams to keep the in-order
PE queue dense while ACT works through the exps.
"""

import os
from collections import deque
from contextlib import ExitStack

import numpy as np
import ml_dtypes

import concourse.bacc as bacc
import concourse.mybir as mybir
import concourse.tile as tile

BF = mybir.dt.bfloat16
F32 = mybir.dt.float32
AFT = mybir.ActivationFunctionType

NCORES = 8
B, T, C = 2, 2048, 4096
H, KV, D = 32, 8, 128
QH = H // NCORES          # 4 q-heads per core
CT = C // 128             # 32 contraction tiles
NCH = T // 512            # 4 query chunks per batch
SCALE = 1.0 / float(np.sqrt(D))
EXP_BIAS = -10.0
ROPE_BASE = 10000.0

bf16 = ml_dtypes.bfloat16


def emit_program():
    nc = bacc.Bacc("TRN2", target_bir_lowering=False, debug=False,
                   num_devices=NCORES)

    xT_d = nc.dram_tensor("xT", [C, B * T], BF, kind="ExternalInput").ap()
    wq_d = nc.dram_tensor("wq", [C, QH * D], BF, kind="ExternalInput").ap()
    wk_d = nc.dram_tensor("wk", [C, D], BF, kind="ExternalInput").ap()
    wv_d = nc.dram_tensor("wv", [C, D], BF, kind="ExternalInput").ap()
    wo_d = nc.dram_tensor("woA", [128, QH, C], BF, kind="ExternalInput").ap()
    cos_d = nc.dram_tensor("cosT", [D, T], BF, kind="ExternalInput").ap()
    sin_d = nc.dram_tensor("sinTr", [D, T], BF, kind="ExternalInput").ap()
    alw_d = nc.dram_tensor("allowA", [128, 4, 512], BF, kind="ExternalInput").ap()
    out_d = nc.dram_tensor("out", [B * T, C], BF, kind="ExternalOutput").ap()

    with tile.TileContext(nc) as tc, ExitStack() as ctx:
        const = ctx.enter_context(tc.tile_pool(name="const", bufs=1))
        act = ctx.enter_context(tc.tile_pool(name="act", bufs=1))
        work = ctx.enter_context(tc.tile_pool(name="work", bufs=1))

        # weights + tables on the gpsimd DMA queue so they never sit ahead of
        # the xt activation loads (sync queue); chunked in 8-c-tile groups so
        # the first projection matmuls wait on ~1.5MB, not the full tensors.
        # cos/sin follow the first group (needed at the first rope evict); the
        # big wo tensor is emitted after P(b0) so it trickles in last.
        wq_sb = const.tile([128, CT, QH * D], BF)
        wk_sb = const.tile([128, CT, D], BF)
        wv_sb = const.tile([128, CT, D], BF)
        wqr = wq_d.rearrange("(ci p) n -> p ci n", p=128)
        wkr = wk_d.rearrange("(ci p) n -> p ci n", p=128)
        wvr = wv_d.rearrange("(ci p) n -> p ci n", p=128)
        cos_sb = const.tile([D, T], BF)
        sin_sb = const.tile([D, T], BF)
        GW = 8
        for g in range(0, CT, GW):
            s = slice(g, g + GW)
            nc.gpsimd.dma_start(wq_sb[:, s, :], wqr[:, s, :])
            nc.gpsimd.dma_start(wk_sb[:, s, :], wkr[:, s, :])
            nc.gpsimd.dma_start(wv_sb[:, s, :], wvr[:, s, :])
            if g == 0:
                nc.gpsimd.dma_start(cos_sb[:], cos_d)
                nc.gpsimd.dma_start(sin_sb[:], sin_d)
        alw_sb = const.tile([128, 4, 512], BF)
        nc.gpsimd.dma_start(alw_sb[:], alw_d)
        wo_sb = const.tile([128, QH, C], BF)
        onesbf_sb = const.tile([128, 128], BF)
        nc.gpsimd.memset(onesbf_sb[:], 1.0)
        bias_sb = const.tile([128, 1], F32)
        nc.gpsimd.memset(bias_sb[:], EXP_BIAS)

        def rope_evict(dst, psum, cs):
            # dst = psum * cos + swap_halves(psum) * sin_rot   (bf16 out)
            sw = work.tile([128, 512], F32, tag="sw", bufs=2, name="sw")
            nc.vector.tensor_copy(sw[0:64, :], psum[64:128, :])
            nc.vector.tensor_copy(sw[64:128, :], psum[0:64, :])
            nc.vector.tensor_mul(sw[:], sw[:], sin_sb[:, cs])
            cst = work.tile([128, 512], F32, tag="cst", bufs=2, name="cst")
            nc.vector.tensor_mul(cst[:], psum[:], cos_sb[:, cs])
            nc.vector.tensor_add(dst, cst[:], sw[:])

        def proj_batch(pp, b):
            qT = act.tile([D, QH, T], BF, tag="qT", bufs=2, name="qT")
            kT = act.tile([D, T], BF, tag="kT", bufs=2, name="kT")
            vT = act.tile([D, T], BF, tag="vT", bufs=2, name="vT")
            vsb = act.tile([128, T // 128, D], BF, tag="v", bufs=2, name="vsb")
            for jc in range(NCH):
                pq = [pp.tile([128, 512], F32, tag=f"pq{h}", name=f"pq{h}")
                      for h in range(QH)]
                pk = pp.tile([128, 512], F32, tag="pk", bufs=2, name="pk")
                pv = pp.tile([128, 512], F32, tag="pv", bufs=2, name="pv")
                # q matmuls run SKEW c-tiles behind k/v so the previous
                # chunk's pq bank evictions are hidden behind ready work
                SKEW = 4
                xts = {}
                col0 = b * T + 512 * jc

                def q_mms(cq):
                    for h in range(QH):
                        nc.tensor.matmul(
                            pq[h][:], wq_sb[:, cq, 128 * h:128 * (h + 1)],
                            xts[cq][:], start=cq == 0, stop=cq == CT - 1)
                    if cq >= SKEW:
                        del xts[cq - SKEW]

                for ci in range(CT):
                    xt = work.tile([128, 512], BF, tag="xt", bufs=10, name="xt")
                    xts[ci] = xt
                    nc.sync.dma_start(
                        xt[:], xT_d[128 * ci:128 * (ci + 1), col0:col0 + 512])
                    st, sp = ci == 0, ci == CT - 1
                    nc.tensor.matmul(pk[:], wk_sb[:, ci, :], xt[:],
                                     start=st, stop=sp)
                    nc.tensor.matmul(pv[:], wv_sb[:, ci, :], xt[:],
                                     start=st, stop=sp)
                    if ci >= SKEW:
                        q_mms(ci - SKEW)
                for cq in range(CT - SKEW, CT):
                    q_mms(cq)
                cs = slice(512 * jc, 512 * (jc + 1))
                nc.scalar.copy(vT[:, cs], pv[:])
                rope_evict(kT[:, cs], pk[:], cs)
                for h in range(QH):
                    rope_evict(qT[:, h, cs], pq[h][:], cs)
                # v chunk -> [t, d] tiles via the DMA crossbar (scalar hwdge
                # queue), off the PE entirely
                for k in range(4 * jc, 4 * jc + 4):
                    nc.scalar.dma_start_transpose(
                        vsb[:, k, :], vT[:, 128 * k:128 * (k + 1)])
            return qT, kT, vsb

        with tc.tile_pool(name="pproj", bufs=1, space="PSUM") as pp:
            acts0 = proj_batch(pp, 0)
            # wo lands behind b0's projection traffic on the gpsimd queue;
            # it is only read in the attention phase
            wor = wo_d
            nc.gpsimd.dma_start(wo_sb[:], wor)
            acts1 = proj_batch(pp, 1)

        # ---- attention + output projection ----
        with tc.tile_pool(name="pattn", bufs=1, space="PSUM") as pa:
            wo_jobs = deque()

            def make_wo_job(b, j, tl, o, yts):
                def job():
                    ops = pa.tile([128, 512], F32, tag="ops", bufs=2,
                                  name="ops")
                    for h in range(QH):
                        nc.tensor.matmul(
                            ops[:], yts[h][:, 128 * tl:128 * (tl + 1)],
                            wo_sb[:, h, 512 * o:512 * (o + 1)],
                            start=h == 0, stop=h == QH - 1)
                    ob = work.tile([128, 512], BF, tag="ob", bufs=4,
                                   name="ob")
                    nc.vector.tensor_copy(ob[:], ops[:])
                    r0 = b * T + 512 * j + 128 * tl
                    nc.sync.dma_start(out_d[r0:r0 + 128, 512 * o:512 * (o + 1)],
                                      ob[:])
                return job

            for b, (qT, kT, vsb) in ((0, acts0), (1, acts1)):
                for j in range(NCH):
                    yts = {}
                    for h in range(QH):
                        yps = pa.tile([128, 512], F32, tag="yps", bufs=1,
                                      name="yps")
                        K = 4 * j + 4
                        # pass 1: score matmuls stream; exp/mask/denominator
                        # trail on ACT/DVE. Diagonal tiles (o>=1) only touch
                        # their unmasked column range [128*o:512].
                        S = work.tile([128, 512], BF, tag="S", bufs=2,
                                      name="S")
                        pts = []
                        for k in range(K):
                            o = k - 4 * j
                            c0 = 128 * o if o > 0 else 0
                            sl = slice(c0, 512)
                            sps = pa.tile([128, 512], F32, tag="sps", bufs=4,
                                          name="sps")
                            nc.tensor.matmul(
                                sps[:, sl], kT[:, 128 * k:128 * (k + 1)],
                                qT[:, h, 512 * j + c0:512 * (j + 1)],
                                start=True, stop=True)
                            pt = work.tile([128, 512], BF, tag="pt", bufs=18,
                                           name="pt")
                            nc.scalar.activation(pt[:, sl], sps[:, sl], AFT.Exp,
                                                 bias=bias_sb[:], scale=SCALE)
                            if o >= 0:
                                nc.vector.tensor_mul(pt[:, sl], pt[:, sl],
                                                     alw_sb[:, o, sl])
                            if k == 0:
                                nc.vector.tensor_copy(S[:], pt[:])
                            else:
                                nc.vector.tensor_add(S[:, sl], S[:, sl],
                                                     pt[:, sl])
                            pts.append((pt, sl))
                            if wo_jobs:
                                wo_jobs.popleft()()
                        # pass 2: attn@v accumulation; k=0 always covers the
                        # full 512 columns so the start-matmul initializes the
                        # whole bank
                        for k, (pt, sl) in enumerate(pts):
                            nc.tensor.matmul(yps[:, sl], vsb[:, k, :],
                                             pt[:, sl],
                                             start=(k == 0), stop=(k == K - 1))
                            if wo_jobs:
                                wo_jobs.popleft()()
                        dns = pa.tile([128, 512], F32, tag="dns", bufs=1,
                                      name="dns")
                        nc.tensor.matmul(dns[:], onesbf_sb[:], S[:],
                                         start=True, stop=True)
                        rec = work.tile([128, 512], F32, tag="rec", bufs=2,
                                        name="rec")
                        nc.vector.reciprocal_approx_fast(rec[:], dns[:])
                        yt = work.tile([128, 512], BF, tag="yt", bufs=8,
                                       name="yt")
                        nc.vector.tensor_mul(yt[:], yps[:], rec[:])
                        yts[h] = yt
                    for tl in range(4):
                        for o in range(C // 512):
                            wo_jobs.append(make_wo_job(b, j, tl, o, yts))
            while wo_jobs:
                wo_jobs.popleft()()

    nc.compile()
    return nc


def host_prep(inputs):
    x = np.asarray(inputs["x"], np.float32)
    mask = np.asarray(inputs["mask"], np.float32)
    wq = np.asarray(inputs["wq"], np.float32)
    wk = np.asarray(inputs["wk"], np.float32)
    wv = np.asarray(inputs["wv"], np.float32)
    wo = np.asarray(inputs["wo"], np.float32)

    xT = np.ascontiguousarray(x.reshape(B * T, C).T).astype(bf16)
    inv = 1.0 / (ROPE_BASE ** (np.arange(0, D, 2, dtype=np.float64) / D))
    freqs = np.arange(T, dtype=np.float64)[:, None] * inv[None, :] * B
    emb = np.concatenate([freqs, freqs], axis=-1)       # [T, D]
    cosT = np.cos(emb).T.astype(np.float32).astype(bf16)
    sinT = np.sin(emb).T.astype(np.float32)
    sinT[: D // 2] *= -1.0
    sinTr = sinT.astype(bf16)
    # allow[p, o, jj] = 1 - mask[jj, 128*o + p]  (from the actual mask input)
    allowA = np.ascontiguousarray(
        np.stack([(1.0 - mask[0:512, 128 * o:128 * (o + 1)]).T
                  for o in range(4)], axis=1)).astype(bf16)   # [128, 4, 512]

    common = dict(xT=xT, cosT=cosT, sinTr=sinTr, allowA=allowA)
    in_maps = []
    for c in range(NCORES):
        m = dict(common)
        m["wq"] = np.ascontiguousarray(wq[:, 512 * c:512 * (c + 1)]).astype(bf16)
        m["wk"] = np.ascontiguousarray(wk[:, 128 * c:128 * (c + 1)]).astype(bf16)
        m["wv"] = np.ascontiguousarray(wv[:, 128 * c:128 * (c + 1)]).astype(bf16)
        m["woA"] = np.ascontiguousarray(
            wo[512 * c:512 * (c + 1), :].reshape(QH, 128, C)
            .transpose(1, 0, 2)).astype(bf16)
        in_maps.append(m)
    return in_maps


def kernel(**inputs) -> np.ndarray:
    from concourse.bass_utils import run_bass_kernel_spmd

    in_maps = host_prep(inputs)
    nc = emit_program()
    trace = bool(os.environ.get("BASS_KERNEL_TRACE"))
    res = run_bass_kernel_spmd(nc, in_maps, core_ids=list(range(NCORES)),
                               trace=trace)
    if trace and res.exec_time_ns is not None:
        print(f"HW exec time: {res.exec_time_ns} ns")
        if res.instructions_and_trace is not None:
            print("trace:", res.instructions_and_trace[1])
    total = np.zeros((B * T, C), np.float32)
    for r in res.results:
        total += np.asarray(r["out"], dtype=np.float32)
    return total.reshape(B, T, C)


# revision 8
# speedup vs baseline: 1.0855x; 1.0523x over previous
"""Trainium2 Bass kernel for GQA attention (B=2, T=2048, C=4096, H=32, KV=8, D=128)
with RoPE and causal mask.

Sharding: tensor-parallel over heads across 8 cores. Each core owns 4 Q heads and
their shared KV head: projects q/k/v for those heads, runs causal attention, and
computes a partial output projection; the host sums the 8 partials (bf16 partials,
f32 accumulation on host).

All on-chip layouts are transposed ([feature, token]) so every matmul consumes
natural slices:
  qT/kT/vT = W^T @ x  via lhsT=W-tile [128c, cols], rhs=xT-tile [128c, 512t]
  sT[tk, tq] = kT-tile^T @ qT-chunk   (per 128-row key tile x 512-col query chunk;
               diagonal tiles stream only their unmasked column range)
  pT = exp(sT/sqrt(D) - 10) on ACT; strictly-causal-upper tiles skipped entirely
  S  = sum_k pT  accumulated on DVE (bf16) -> one ones-matmul per (b,h,j) gives
       the softmax denominator broadcast in PSUM (replaces a ones-matmul per tile)
  yT[d, tq] += v-tile^T @ pT          (v laid out [t, d] via DMA-crossbar transpose)
  out[tq, :] += yT_h^T @ wo_h         (accumulate 4 heads in PSUM, evict bf16, DMA)

Phase order is P(b0) P(b1) A(b0) A(b1) with double-buffered qT/kT/vsb so the PE
never sees a projection<->attention boundary stall. Output-projection matmul
"jobs" are popped from a queue inside the attention streams to keep the in-order
PE queue dense while ACT works through the exps.
"""

import os
from collections import deque
from contextlib import ExitStack

import numpy as np
import ml_dtypes

import concourse.bacc as bacc
import concourse.mybir as mybir
import concourse.tile as tile

BF = mybir.dt.bfloat16
F32 = mybir.dt.float32
AFT = mybir.ActivationFunctionType

NCORES = 8
B, T, C = 2, 2048, 4096
H, KV, D = 32, 8, 128
QH = H // NCORES          # 4 q-heads per core
CT = C // 128             # 32 contraction tiles
NCH = T // 512            # 4 query chunks per batch
SCALE = 1.0 / float(np.sqrt(D))
EXP_BIAS = -10.0
ROPE_BASE = 10000.0

bf16 = ml_dtypes.bfloat16


def emit_program():
    nc = bacc.Bacc("TRN2", target_bir_lowering=False, debug=False,
                   num_devices=NCORES)

    xT_d = nc.dram_tensor("xT", [C, B * T], BF, kind="ExternalInput").ap()
    wq_d = nc.dram_tensor("wq", [C, QH * D], BF, kind="ExternalInput").ap()
    wk_d = nc.dram_tensor("wk", [C, D], BF, kind="ExternalInput").ap()
    wv_d = nc.dram_tensor("wv", [C, D], BF, kind="ExternalInput").ap()
    wo_d = nc.dram_tensor("woA", [128, QH, C], BF, kind="ExternalInput").ap()
    cos_d = nc.dram_tensor("cosT", [D, T], BF, kind="ExternalInput").ap()
    sin_d = nc.dram_tensor("sinTr", [D, T], BF, kind="ExternalInput").ap()
    alw_d = nc.dram_tensor("allowA", [128, 4, 512], BF, kind="ExternalInput").ap()
    out_d = nc.dram_tensor("out", [B * T, C], BF, kind="ExternalOutput").ap()

    with tile.TileContext(nc) as tc, ExitStack() as ctx:
        const = ctx.enter_context(tc.tile_pool(name="const", bufs=1))
        act = ctx.enter_context(tc.tile_pool(name="act", bufs=1))
        work = ctx.enter_context(tc.tile_pool(name="work", bufs=1))

        # weights + tables on the gpsimd DMA queue so they never sit ahead of
        # the xt activation loads (sync queue); chunked in 8-c-tile groups so
        # the first projection matmuls wait on ~1.5MB, not the full tensors.
        # cos/sin follow the first group (needed at the first rope evict); the
        # big wo tensor is emitted after P(b0) so it trickles in last.
        wq_sb = const.tile([128, CT, QH * D], BF)
        wk_sb = const.tile([128, CT, D], BF)
        wv_sb = const.tile([128, CT, D], BF)
        wqr = wq_d.rearrange("(ci p) n -> p ci n", p=128)
        wkr = wk_d.rearrange("(ci p) n -> p ci n", p=128)
        wvr = wv_d.rearrange("(ci p) n -> p ci n", p=128)
        cos_sb = const.tile([D, T], BF)
        sin_sb = const.tile([D, T], BF)
        GW = 8
        for g in range(0, CT, GW):
            s = slice(g, g + GW)
            nc.gpsimd.dma_start(wq_sb[:, s, :], wqr[:, s, :])
            nc.gpsimd.dma_start(wk_sb[:, s, :], wkr[:, s, :])
            nc.gpsimd.dma_start(wv_sb[:, s, :], wvr[:, s, :])
            if g == GW:
                nc.gpsimd.dma_start(cos_sb[:], cos_d)
                nc.gpsimd.dma_start(sin_sb[:], sin_d)
        alw_sb = const.tile([128, 4, 512], BF)
        nc.gpsimd.dma_start(alw_sb[:], alw_d)
        wo_sb = const.tile([128, QH, C], BF)
        onesbf_sb = const.tile([128, 128], BF)
        nc.gpsimd.memset(onesbf_sb[:], 1.0)
        bias_sb = const.tile([128, 1], F32)
        nc.gpsimd.memset(bias_sb[:], EXP_BIAS)

        def rope_sb(dst, src, cs):
            # dst = src * cos + swap_halves(src) * sin_rot   (all bf16 SBUF so
            # DVE runs in 2x/4x perf modes; src was evicted from PSUM by ACT)
            sw = work.tile([128, 512], BF, tag="sw", bufs=2, name="sw")
            nc.vector.tensor_copy(sw[0:64, :], src[64:128, :])
            nc.vector.tensor_copy(sw[64:128, :], src[0:64, :])
            nc.vector.tensor_mul(sw[:], sw[:], sin_sb[:, cs])
            cst = work.tile([128, 512], BF, tag="cst", bufs=2, name="cst")
            nc.vector.tensor_mul(cst[:], src[:], cos_sb[:, cs])
            nc.vector.tensor_add(dst, cst[:], sw[:])

        def proj_batch(pp, b):
            qT = act.tile([D, QH, T], BF, tag="qT", bufs=2, name="qT")
            kT = act.tile([D, T], BF, tag="kT", bufs=2, name="kT")
            vT = act.tile([D, T], BF, tag="vT", bufs=2, name="vT")
            vsb = act.tile([128, T // 128, D], BF, tag="v", bufs=2, name="vsb")
            for jc in range(NCH):
                pq = [pp.tile([128, 512], F32, tag=f"pq{h}", name=f"pq{h}")
                      for h in range(QH)]
                pk = pp.tile([128, 512], F32, tag="pk", bufs=2, name="pk")
                pv = pp.tile([128, 512], F32, tag="pv", bufs=2, name="pv")
                # q matmuls run SKEW c-tiles behind k/v so the previous
                # chunk's pq bank evictions are hidden behind ready work
                SKEW = 4
                xts = {}
                col0 = b * T + 512 * jc

                def q_mms(cq):
                    for h in range(QH):
                        nc.tensor.matmul(
                            pq[h][:], wq_sb[:, cq, 128 * h:128 * (h + 1)],
                            xts[cq][:], start=cq == 0, stop=cq == CT - 1)
                    if cq >= SKEW:
                        del xts[cq - SKEW]

                for ci in range(CT):
                    xt = work.tile([128, 512], BF, tag="xt", bufs=10, name="xt")
                    xts[ci] = xt
                    nc.sync.dma_start(
                        xt[:], xT_d[128 * ci:128 * (ci + 1), col0:col0 + 512])
                    st, sp = ci == 0, ci == CT - 1
                    nc.tensor.matmul(pk[:], wk_sb[:, ci, :], xt[:],
                                     start=st, stop=sp)
                    nc.tensor.matmul(pv[:], wv_sb[:, ci, :], xt[:],
                                     start=st, stop=sp)
                    if ci >= SKEW:
                        q_mms(ci - SKEW)
                for cq in range(CT - SKEW, CT):
                    q_mms(cq)
                cs = slice(512 * jc, 512 * (jc + 1))
                # fast ACT copies free the PSUM banks within ~3.5us so the
                # next chunk's matmuls never wait on the rope math; the rope
                # itself runs SBUF-side on DVE afterwards
                kraw = work.tile([128, 512], BF, tag="kraw", bufs=2,
                                 name="kraw")
                nc.scalar.copy(kraw[:], pk[:])
                qraws = []
                for h in range(QH):
                    qraw = work.tile([128, 512], BF, tag="qraw", bufs=6,
                                     name="qraw")
                    nc.scalar.copy(qraw[:], pq[h][:])
                    qraws.append(qraw)
                nc.scalar.copy(vT[:, cs], pv[:])
                rope_sb(kT[:, cs], kraw, cs)
                for h in range(QH):
                    rope_sb(qT[:, h, cs], qraws[h], cs)
                # v chunk -> [t, d] tiles via the DMA crossbar (scalar hwdge
                # queue), off the PE entirely
                for k in range(4 * jc, 4 * jc + 4):
                    nc.scalar.dma_start_transpose(
                        vsb[:, k, :], vT[:, 128 * k:128 * (k + 1)])
            return qT, kT, vsb

        with tc.tile_pool(name="pproj", bufs=1, space="PSUM") as pp:
            acts0 = proj_batch(pp, 0)
            # wo lands behind b0's projection traffic on the gpsimd queue;
            # it is only read in the attention phase
            wor = wo_d
            nc.gpsimd.dma_start(wo_sb[:], wor)
            acts1 = proj_batch(pp, 1)

        # ---- attention + output projection ----
        with tc.tile_pool(name="pattn", bufs=1, space="PSUM") as pa:
            wo_jobs = deque()

            def make_wo_job(b, j, tl, o, yts):
                def job():
                    ops = pa.tile([128, 512], F32, tag="ops", bufs=2,
                                  name="ops")
                    for h in range(QH):
                        nc.tensor.matmul(
                            ops[:], yts[h][:, 128 * tl:128 * (tl + 1)],
                            wo_sb[:, h, 512 * o:512 * (o + 1)],
                            start=h == 0, stop=h == QH - 1)
                    ob = work.tile([128, 512], BF, tag="ob", bufs=4,
                                   name="ob")
                    # alternate the PSUM eviction between ACT and DVE so
                    # neither sidecar engine becomes the bottleneck
                    if o % 2 == 0:
                        nc.scalar.copy(ob[:], ops[:])
                    else:
                        nc.vector.tensor_copy(ob[:], ops[:])
                    r0 = b * T + 512 * j + 128 * tl
                    nc.sync.dma_start(out_d[r0:r0 + 128, 512 * o:512 * (o + 1)],
                                      ob[:])
                return job

            for b, (qT, kT, vsb) in ((0, acts0), (1, acts1)):
                for j in range(NCH):
                    yts = {}
                    for h in range(QH):
                        yps = pa.tile([128, 512], F32, tag="yps", bufs=1,
                                      name="yps")
                        K = 4 * j + 4
                        # pass 1: score matmuls stream; exp/mask/denominator
                        # trail on ACT/DVE. Diagonal tiles (o>=1) only touch
                        # their unmasked column range [128*o:512].
                        S = work.tile([128, 512], BF, tag="S", bufs=2,
                                      name="S")
                        pts = []
                        for k in range(K):
                            o = k - 4 * j
                            c0 = 128 * o if o > 0 else 0
                            sl = slice(c0, 512)
                            sps = pa.tile([128, 512], F32, tag="sps", bufs=4,
                                          name="sps")
                            nc.tensor.matmul(
                                sps[:, sl], kT[:, 128 * k:128 * (k + 1)],
                                qT[:, h, 512 * j + c0:512 * (j + 1)],
                                start=True, stop=True)
                            pt = work.tile([128, 512], BF, tag="pt", bufs=16,
                                           name="pt")
                            nc.scalar.activation(pt[:, sl], sps[:, sl], AFT.Exp,
                                                 bias=bias_sb[:], scale=SCALE)
                            if o >= 0:
                                nc.vector.tensor_mul(pt[:, sl], pt[:, sl],
                                                     alw_sb[:, o, sl])
                            if k == 0:
                                nc.vector.tensor_copy(S[:], pt[:])
                            else:
                                nc.vector.tensor_add(S[:, sl], S[:, sl],
                                                     pt[:, sl])
                            pts.append((pt, sl))
                            if wo_jobs:
                                wo_jobs.popleft()()
                        # pass 2: attn@v accumulation; k=0 always covers the
                        # full 512 columns so the start-matmul initializes the
                        # whole bank
                        for k, (pt, sl) in enumerate(pts):
                            nc.tensor.matmul(yps[:, sl], vsb[:, k, :],
                                             pt[:, sl],
                                             start=(k == 0), stop=(k == K - 1))
                            if wo_jobs:
                                wo_jobs.popleft()()
                        dns = pa.tile([128, 512], F32, tag="dns", bufs=1,
                                      name="dns")
                        nc.tensor.matmul(dns[:], onesbf_sb[:], S[:],
                                         start=True, stop=True)
                        rec = work.tile([128, 512], F32, tag="rec", bufs=2,
                                        name="rec")
                        nc.vector.reciprocal_approx_fast(rec[:], dns[:])
                        yt = work.tile([128, 512], BF, tag="yt", bufs=8,
                                       name="yt")
                        nc.vector.tensor_mul(yt[:], yps[:], rec[:])
                        yts[h] = yt
                    for tl in range(4):
                        for o in range(C // 512):
                            wo_jobs.append(make_wo_job(b, j, tl, o, yts))
            while wo_jobs:
                wo_jobs.popleft()()

    nc.compile()
    return nc


def host_prep(inputs):
    x = np.asarray(inputs["x"], np.float32)
    mask = np.asarray(inputs["mask"], np.float32)
    wq = np.asarray(inputs["wq"], np.float32)
    wk = np.asarray(inputs["wk"], np.float32)
    wv = np.asarray(inputs["wv"], np.float32)
    wo = np.asarray(inputs["wo"], np.float32)

    xT = np.ascontiguousarray(x.reshape(B * T, C).T).astype(bf16)
    inv = 1.0 / (ROPE_BASE ** (np.arange(0, D, 2, dtype=np.float64) / D))
    freqs = np.arange(T, dtype=np.float64)[:, None] * inv[None, :] * B
    emb = np.concatenate([freqs, freqs], axis=-1)       # [T, D]
    cosT = np.cos(emb).T.astype(np.float32).astype(bf16)
    sinT = np.sin(emb).T.astype(np.float32)
    sinT[: D // 2] *= -1.0
    sinTr = sinT.astype(bf16)
    # allow[p, o, jj] = 1 - mask[jj, 128*o + p]  (from the actual mask input)
    allowA = np.ascontiguousarray(
        np.stack([(1.0 - mask[0:512, 128 * o:128 * (o + 1)]).T
                  for o in range(4)], axis=1)).astype(bf16)   # [128, 4, 512]

    common = dict(xT=xT, cosT=cosT, sinTr=sinTr, allowA=allowA)
    in_maps = []
    for c in range(NCORES):
        m = dict(common)
        m["wq"] = np.ascontiguousarray(wq[:, 512 * c:512 * (c + 1)]).astype(bf16)
        m["wk"] = np.ascontiguousarray(wk[:, 128 * c:128 * (c + 1)]).astype(bf16)
        m["wv"] = np.ascontiguousarray(wv[:, 128 * c:128 * (c + 1)]).astype(bf16)
        m["woA"] = np.ascontiguousarray(
            wo[512 * c:512 * (c + 1), :].reshape(QH, 128, C)
            .transpose(1, 0, 2)).astype(bf16)
        in_maps.append(m)
    return in_maps


def kernel(**inputs) -> np.ndarray:
    from concourse.bass_utils import run_bass_kernel_spmd

    in_maps = host_prep(inputs)
    nc = emit_program()
    trace = bool(os.environ.get("BASS_KERNEL_TRACE"))
    res = run_bass_kernel_spmd(nc, in_maps, core_ids=list(range(NCORES)),
                               trace=trace)
    if trace and res.exec_time_ns is not None:
        print(f"HW exec time: {res.exec_time_ns} ns")
        if res.instructions_and_trace is not None:
            print("trace:", res.instructions_and_trace[1])
    total = np.zeros((B * T, C), np.float32)
    for r in res.results:
        total += np.asarray(r["out"], dtype=np.float32)
    return total.reshape(B, T, C)


# revision 14
# speedup vs baseline: 1.2486x; 1.1502x over previous
"""Trainium2 Bass kernel for GQA attention (B=2, T=2048, C=4096, H=32, KV=8, D=128)
with RoPE and causal mask.

Sharding: tensor-parallel over heads across 8 cores. Each core owns 4 Q heads and
their shared KV head: projects q/k/v for those heads, runs causal attention, and
computes a partial output projection; the host sums the 8 partials (bf16 partials,
f32 accumulation on host).

All on-chip layouts are transposed ([feature, token]) so every matmul consumes
natural slices:
  qT/kT/vT = W^T @ x  via lhsT=W-tile [128c, cols], rhs=xT-tile [128c, 512t]
  sT[tk, tq] = kT-tile^T @ qT-chunk   (per 128-row key tile x 512-col query chunk;
               diagonal tiles stream only their unmasked column range)
  pT = exp(sT/sqrt(D) - 10) on ACT; strictly-causal-upper tiles skipped entirely
  S  = sum_k pT  accumulated on DVE (bf16) -> one ones-matmul per (b,h,j) gives
       the softmax denominator broadcast in PSUM (replaces a ones-matmul per tile)
  yT[d, tq] += v-tile^T @ pT          (v laid out [t, d] via DMA-crossbar transpose)
  out[tq, :] += yT_h^T @ wo_h         (accumulate 4 heads in PSUM, evict bf16, DMA)

Phase order is P(b0) P(b1) A(b0) A(b1) with double-buffered qT/kT/vsb so the PE
never sees a projection<->attention boundary stall. Output-projection matmul
"jobs" are popped from a queue inside the attention streams to keep the in-order
PE queue dense while ACT works through the exps.
"""

import os
from collections import deque
from contextlib import ExitStack

import numpy as np
import ml_dtypes

import concourse.bacc as bacc
import concourse.mybir as mybir
import concourse.tile as tile

BF = mybir.dt.bfloat16
F32 = mybir.dt.float32
AFT = mybir.ActivationFunctionType

NCORES = 8
B, T, C = 2, 2048, 4096
H, KV, D = 32, 8, 128
QH = H // NCORES          # 4 q-heads per core
CT = C // 128             # 32 contraction tiles
NCH = T // 512            # 4 query chunks per batch
SCALE = 1.0 / float(np.sqrt(D))
EXP_BIAS = -10.0
ROPE_BASE = 10000.0

bf16 = ml_dtypes.bfloat16


def emit_program():
    nc = bacc.Bacc("TRN2", target_bir_lowering=False, debug=False,
                   num_devices=NCORES)

    xT_d = nc.dram_tensor("xT", [C, B * T], BF, kind="ExternalInput").ap()
    wq_d = nc.dram_tensor("wq", [C, QH * D], BF, kind="ExternalInput").ap()
    wk_d = nc.dram_tensor("wk", [C, D], BF, kind="ExternalInput").ap()
    wv_d = nc.dram_tensor("wv", [C, D], BF, kind="ExternalInput").ap()
    wo_d = nc.dram_tensor("woA", [128, QH, C], BF, kind="ExternalInput").ap()
    cos_d = nc.dram_tensor("cosT", [D, T], BF, kind="ExternalInput").ap()
    sin_d = nc.dram_tensor("sinTr", [D, T], BF, kind="ExternalInput").ap()
    alw_d = nc.dram_tensor("allowA", [128, 4, 512], BF, kind="ExternalInput").ap()
    out_d = nc.dram_tensor("out", [B * T, C], BF, kind="ExternalOutput").ap()

    with tile.TileContext(nc) as tc, ExitStack() as ctx:
        const = ctx.enter_context(tc.tile_pool(name="const", bufs=1))
        act = ctx.enter_context(tc.tile_pool(name="act", bufs=1))
        work = ctx.enter_context(tc.tile_pool(name="work", bufs=1))

        # weights + tables on the gpsimd DMA queue so they never sit ahead of
        # the xt activation loads (sync queue); chunked in 8-c-tile groups so
        # the first projection matmuls wait on ~1.5MB, not the full tensors.
        # cos/sin follow the first group (needed at the first rope evict); the
        # big wo tensor is emitted after P(b0) so it trickles in last.
        wq_sb = const.tile([128, CT, QH * D], BF)
        wk_sb = const.tile([128, CT, D], BF)
        wv_sb = const.tile([128, CT, D], BF)
        wqr = wq_d.rearrange("(ci p) n -> p ci n", p=128)
        wkr = wk_d.rearrange("(ci p) n -> p ci n", p=128)
        wvr = wv_d.rearrange("(ci p) n -> p ci n", p=128)
        cos_sb = const.tile([D, T], BF)
        sin_sb = const.tile([D, T], BF)
        GW = 8
        for g in range(0, CT, GW):
            s = slice(g, g + GW)
            nc.gpsimd.dma_start(wq_sb[:, s, :], wqr[:, s, :])
            nc.gpsimd.dma_start(wk_sb[:, s, :], wkr[:, s, :])
            nc.gpsimd.dma_start(wv_sb[:, s, :], wvr[:, s, :])
            if g == GW:
                nc.gpsimd.dma_start(cos_sb[:], cos_d)
                nc.gpsimd.dma_start(sin_sb[:], sin_d)
        alw_sb = const.tile([128, 4, 512], BF)
        nc.gpsimd.dma_start(alw_sb[:], alw_d)
        wo_sb = const.tile([128, QH, C], BF)
        onesbf_sb = const.tile([128, 128], BF)
        nc.gpsimd.memset(onesbf_sb[:], 1.0)
        bias_sb = const.tile([128, 1], F32)
        nc.gpsimd.memset(bias_sb[:], EXP_BIAS)

        def rope_sb(dst, src, cs):
            # dst = src * cos + swap_halves(src) * sin_rot   (all bf16 SBUF so
            # DVE runs in 2x/4x perf modes; src was evicted from PSUM by ACT)
            sw = work.tile([128, 512], BF, tag="sw", bufs=2, name="sw")
            nc.vector.tensor_copy(sw[0:64, :], src[64:128, :])
            nc.vector.tensor_copy(sw[64:128, :], src[0:64, :])
            nc.vector.tensor_mul(sw[:], sw[:], sin_sb[:, cs])
            cst = work.tile([128, 512], BF, tag="cst", bufs=2, name="cst")
            nc.vector.tensor_mul(cst[:], src[:], cos_sb[:, cs])
            nc.vector.tensor_add(dst, cst[:], sw[:])

        tr_pending = deque()

        def flush_tr(n):
            for _ in range(min(n, len(tr_pending))):
                tr_pending.popleft()()

        def proj_batch(pp, b):
            qT = act.tile([D, QH, T], BF, tag="qT", bufs=2, name="qT")
            kT = act.tile([D, T], BF, tag="kT", bufs=2, name="kT")
            vT = act.tile([D, T], BF, tag="vT", bufs=2, name="vT")
            vsb = act.tile([128, T // 128, D], BF, tag="v", bufs=2, name="vsb")
            for jc in range(NCH):
                # previous chunk's v transposes go on the ACT queue here, so
                # they sit between that chunk's bank-freeing copies and this
                # chunk's (the ACT queue is idle mid-chunk)
                flush_tr(4)
                pq = [pp.tile([128, 512], F32, tag=f"pq{h}", name=f"pq{h}")
                      for h in range(QH)]
                pk = pp.tile([128, 512], F32, tag="pk", bufs=2, name="pk")
                pv = pp.tile([128, 512], F32, tag="pv", bufs=2, name="pv")
                # q matmuls run SKEW c-tiles behind k/v so the previous
                # chunk's pq bank evictions are hidden behind ready work
                SKEW = 4
                xts = {}
                col0 = b * T + 512 * jc

                def q_mms(cq, h):
                    nc.tensor.matmul(
                        pq[h][:], wq_sb[:, cq, 128 * h:128 * (h + 1)],
                        xts[cq][:], start=cq == 0, stop=cq == CT - 1)

                for ci in range(CT):
                    xt = work.tile([128, 512], BF, tag="xt", bufs=12, name="xt")
                    xts[ci] = xt
                    nc.sync.dma_start(
                        xt[:], xT_d[128 * ci:128 * (ci + 1), col0:col0 + 512])
                    st, sp = ci == 0, ci == CT - 1
                    nc.tensor.matmul(pk[:], wk_sb[:, ci, :], xt[:],
                                     start=st, stop=sp)
                    nc.tensor.matmul(pv[:], wv_sb[:, ci, :], xt[:],
                                     start=st, stop=sp)
                    if ci >= SKEW:
                        for h in range(QH):
                            q_mms(ci - SKEW, h)
                        del xts[ci - SKEW]
                cs = slice(512 * jc, 512 * (jc + 1))
                # fast ACT copies free the PSUM banks so the next chunk's
                # matmuls never wait on the rope math (which runs SBUF-side
                # on DVE afterwards); kraw/vT copies overlap the q tail, and
                # the head-major tail staggers the pq stops so the qraw
                # copies pipeline against remaining tail matmuls
                kraw = work.tile([128, 512], BF, tag="kraw", bufs=2,
                                 name="kraw")
                nc.scalar.copy(kraw[:], pk[:])
                nc.scalar.copy(vT[:, cs], pv[:])
                qraws = []
                for h in range(QH):
                    for cq in range(CT - SKEW, CT):
                        q_mms(cq, h)
                    qraw = work.tile([128, 512], BF, tag="qraw", bufs=6,
                                     name="qraw")
                    nc.scalar.copy(qraw[:], pq[h][:])
                    qraws.append(qraw)
                rope_sb(kT[:, cs], kraw, cs)
                for h in range(QH):
                    rope_sb(qT[:, h, cs], qraws[h], cs)
                # v chunk -> [t, d] tiles via the DMA crossbar (scalar hwdge
                # queue), off the PE entirely; deferred one chunk
                for k in range(4 * jc, 4 * jc + 4):
                    tr_pending.append(
                        lambda k=k, vsb=vsb, vT=vT: nc.scalar.dma_start_transpose(
                            vsb[:, k, :], vT[:, 128 * k:128 * (k + 1)]))
            return qT, kT, vsb

        with tc.tile_pool(name="pproj", bufs=1, space="PSUM") as pp:
            acts0 = proj_batch(pp, 0)
            # gate the wo load on a DVE op emitted here so the DMA cannot
            # start before P(b0) finishes -- keeps the HBM free for the xt
            # stream during the warmup chunks (wo is only read in attention)
            nc.vector.memset(wo_sb[:, 0, 0:8], 0.0)
            nc.gpsimd.dma_start(wo_sb[:], wo_d)
            acts1 = proj_batch(pp, 1)

        # ---- attention + output projection ----
        with tc.tile_pool(name="pattn", bufs=1, space="PSUM") as pa:
            wo_jobs = deque()

            def make_wo_job(b, j, tl, op, yts):
                # one job covers two adjacent 512-col output slices so the
                # store DMA gets 2KB lines (one [128,1024] bf16 transfer)
                def job():
                    ob = work.tile([128, 1024], BF, tag="ob", bufs=3,
                                   name="ob")
                    for half in range(2):
                        o = 2 * op + half
                        ops = pa.tile([128, 512], F32, tag="ops", bufs=2,
                                      name="ops")
                        for h in range(QH):
                            nc.tensor.matmul(
                                ops[:], yts[h][:, 128 * tl:128 * (tl + 1)],
                                wo_sb[:, h, 512 * o:512 * (o + 1)],
                                start=h == 0, stop=h == QH - 1)
                        # alternate the PSUM eviction between ACT and DVE so
                        # neither sidecar engine becomes the bottleneck
                        if half == 0:
                            nc.scalar.copy(ob[:, 0:512], ops[:])
                        else:
                            nc.vector.tensor_copy(ob[:, 512:1024], ops[:])
                    r0 = b * T + 512 * j + 128 * tl
                    nc.sync.dma_start(
                        out_d[r0:r0 + 128, 1024 * op:1024 * (op + 1)], ob[:])
                return job

            for b, (qT, kT, vsb) in ((0, acts0), (1, acts1)):
                for j in range(NCH):
                    yts = {}
                    for h in range(QH):
                        # sprinkle the last projection chunk's v transposes
                        # into the early attention heads (ACT queue slack)
                        flush_tr(1)
                        yps = pa.tile([128, 512], F32, tag="yps", bufs=1,
                                      name="yps")
                        K = 4 * j + 4
                        # pass 1: score matmuls stream; exp/mask/denominator
                        # trail on ACT/DVE. Diagonal tiles (o>=1) only touch
                        # their unmasked column range [128*o:512].
                        S = work.tile([128, 512], BF, tag="S", bufs=2,
                                      name="S")
                        pts = []
                        for k in range(K):
                            o = k - 4 * j
                            c0 = 128 * o if o > 0 else 0
                            sl = slice(c0, 512)
                            sps = pa.tile([128, 512], F32, tag="sps", bufs=4,
                                          name="sps")
                            nc.tensor.matmul(
                                sps[:, sl], kT[:, 128 * k:128 * (k + 1)],
                                qT[:, h, 512 * j + c0:512 * (j + 1)],
                                start=True, stop=True)
                            pt = work.tile([128, 512], BF, tag="pt", bufs=16,
                                           name="pt")
                            nc.scalar.activation(pt[:, sl], sps[:, sl], AFT.Exp,
                                                 bias=bias_sb[:], scale=SCALE)
                            if o >= 0:
                                nc.vector.tensor_mul(pt[:, sl], pt[:, sl],
                                                     alw_sb[:, o, sl])
                            if k == 0:
                                nc.vector.tensor_copy(S[:], pt[:])
                            else:
                                nc.vector.tensor_add(S[:, sl], S[:, sl],
                                                     pt[:, sl])
                            pts.append((pt, sl))
                            if wo_jobs:
                                wo_jobs.popleft()()
                        # pass 2: attn@v accumulation; k=0 always covers the
                        # full 512 columns so the start-matmul initializes the
                        # whole bank
                        for k, (pt, sl) in enumerate(pts):
                            nc.tensor.matmul(yps[:, sl], vsb[:, k, :],
                                             pt[:, sl],
                                             start=(k == 0), stop=(k == K - 1))
                            if wo_jobs:
                                wo_jobs.popleft()()
                        dns = pa.tile([128, 512], F32, tag="dns", bufs=1,
                                      name="dns")
                        nc.tensor.matmul(dns[:], onesbf_sb[:], S[:],
                                         start=True, stop=True)
                        rec = work.tile([128, 512], F32, tag="rec", bufs=1,
                                        name="rec")
                        nc.vector.reciprocal_approx_fast(rec[:], dns[:])
                        yt = work.tile([128, 512], BF, tag="yt", bufs=8,
                                       name="yt")
                        nc.vector.tensor_mul(yt[:], yps[:], rec[:])
                        yts[h] = yt
                    for tl in range(4):
                        for op in range(C // 1024):
                            wo_jobs.append(make_wo_job(b, j, tl, op, yts))
            while wo_jobs:
                wo_jobs.popleft()()

    nc.compile()
    return nc


def host_prep(inputs):
    x = np.asarray(inputs["x"], np.float32)
    mask = np.asarray(inputs["mask"], np.float32)
    wq = np.asarray(inputs["wq"], np.float32)
    wk = np.asarray(inputs["wk"], np.float32)
    wv = np.asarray(inputs["wv"], np.float32)
    wo = np.asarray(inputs["wo"], np.float32)

    xT = np.ascontiguousarray(x.reshape(B * T, C).T).astype(bf16)
    inv = 1.0 / (ROPE_BASE ** (np.arange(0, D, 2, dtype=np.float64) / D))
    freqs = np.arange(T, dtype=np.float64)[:, None] * inv[None, :] * B
    emb = np.concatenate([freqs, freqs], axis=-1)       # [T, D]
    cosT = np.cos(emb).T.astype(np.float32).astype(bf16)
    sinT = np.sin(emb).T.astype(np.float32)
    sinT[: D // 2] *= -1.0
    sinTr = sinT.astype(bf16)
    # allow[p, o, jj] = 1 - mask[jj, 128*o + p]  (from the actual mask input)
    allowA = np.ascontiguousarray(
        np.stack([(1.0 - mask[0:512, 128 * o:128 * (o + 1)]).T
                  for o in range(4)], axis=1)).astype(bf16)   # [128, 4, 512]

    common = dict(xT=xT, cosT=cosT, sinTr=sinTr, allowA=allowA)
    in_maps = []
    for c in range(NCORES):
        m = dict(common)
        m["wq"] = np.ascontiguousarray(wq[:, 512 * c:512 * (c + 1)]).astype(bf16)
        m["wk"] = np.ascontiguousarray(wk[:, 128 * c:128 * (c + 1)]).astype(bf16)
        m["wv"] = np.ascontiguousarray(wv[:, 128 * c:128 * (c + 1)]).astype(bf16)
        m["woA"] = np.ascontiguousarray(
            wo[512 * c:512 * (c + 1), :].reshape(QH, 128, C)
            .transpose(1, 0, 2)).astype(bf16)
        in_maps.append(m)
    return in_maps


def kernel(**inputs) -> np.ndarray:
    from concourse.bass_utils import run_bass_kernel_spmd

    in_maps = host_prep(inputs)
    nc = emit_program()
    trace = bool(os.environ.get("BASS_KERNEL_TRACE"))
    res = run_bass_kernel_spmd(nc, in_maps, core_ids=list(range(NCORES)),
                               trace=trace)
    if trace and res.exec_time_ns is not None:
        print(f"HW exec time: {res.exec_time_ns} ns")
        if res.instructions_and_trace is not None:
            print("trace:", res.instructions_and_trace[1])
    total = np.zeros((B * T, C), np.float32)
    for r in res.results:
        total += np.asarray(r["out"], dtype=np.float32)
    return total.reshape(B, T, C)


# revision 15
# speedup vs baseline: 1.3468x; 1.0787x over previous
"""Trainium2 Bass kernel for GQA attention (B=2, T=2048, C=4096, H=32, KV=8, D=128)
with RoPE and causal mask.

Sharding: tensor-parallel over heads across 8 cores. Each core owns 4 Q heads and
their shared KV head: projects q/k/v for those heads, runs causal attention, and
computes a partial output projection; the host sums the 8 partials (bf16 partials,
f32 accumulation on host).

All on-chip layouts are transposed ([feature, token]) so every matmul consumes
natural slices:
  qT/kT/vT = W^T @ x  via lhsT=W-tile [128c, cols], rhs=xT-tile [128c, 512t]
  sT[tk, tq] = kT-tile^T @ qT-chunk   (per 128-row key tile x 512-col query chunk;
               diagonal tiles stream only their unmasked column range)
  pT = exp(sT/sqrt(D) - 10) on ACT; strictly-causal-upper tiles skipped entirely
  S  = sum_k pT  accumulated on DVE (bf16) -> one ones-matmul per (b,h,j) gives
       the softmax denominator broadcast in PSUM (replaces a ones-matmul per tile)
  yT[d, tq] += v-tile^T @ pT          (v laid out [t, d] via DMA-crossbar transpose)
  out[tq, :] += yT_h^T @ wo_h         (accumulate 4 heads in PSUM, evict bf16, DMA)

Phase order is P(b0) P(b1) A(b0) A(b1) with double-buffered qT/kT/vsb so the PE
never sees a projection<->attention boundary stall. Output-projection matmul
"jobs" are popped from a queue inside the attention streams to keep the in-order
PE queue dense while ACT works through the exps.
"""

import os
from collections import deque
from contextlib import ExitStack

import numpy as np
import ml_dtypes

import concourse.bacc as bacc
import concourse.mybir as mybir
import concourse.tile as tile

BF = mybir.dt.bfloat16
F32 = mybir.dt.float32
AFT = mybir.ActivationFunctionType

NCORES = 8
B, T, C = 2, 2048, 4096
H, KV, D = 32, 8, 128
QH = H // NCORES          # 4 q-heads per core
CT = C // 128             # 32 contraction tiles
NCH = T // 512            # 4 query chunks per batch
SCALE = 1.0 / float(np.sqrt(D))
EXP_BIAS = -10.0
ROPE_BASE = 10000.0

bf16 = ml_dtypes.bfloat16


def emit_program():
    nc = bacc.Bacc("TRN2", target_bir_lowering=False, debug=False,
                   num_devices=NCORES)

    xT_d = nc.dram_tensor("xT", [C, B * T], BF, kind="ExternalInput").ap()
    wq_d = nc.dram_tensor("wq", [C, QH * D], BF, kind="ExternalInput").ap()
    wk_d = nc.dram_tensor("wk", [C, D], BF, kind="ExternalInput").ap()
    wv_d = nc.dram_tensor("wv", [C, D], BF, kind="ExternalInput").ap()
    wo_d = nc.dram_tensor("woA", [128, QH, C], BF, kind="ExternalInput").ap()
    cos_d = nc.dram_tensor("cosT", [D, T], BF, kind="ExternalInput").ap()
    sin_d = nc.dram_tensor("sinTr", [D, T], BF, kind="ExternalInput").ap()
    alw_d = nc.dram_tensor("allowA", [128, 4, 512], BF, kind="ExternalInput").ap()
    out_d = nc.dram_tensor("out", [B * T, C], BF, kind="ExternalOutput").ap()

    with tile.TileContext(nc) as tc, ExitStack() as ctx:
        const = ctx.enter_context(tc.tile_pool(name="const", bufs=1))
        act = ctx.enter_context(tc.tile_pool(name="act", bufs=1))
        work = ctx.enter_context(tc.tile_pool(name="work", bufs=1))

        # weights + tables on the gpsimd DMA queue so they never sit ahead of
        # the xt activation loads (sync queue); chunked in 8-c-tile groups so
        # the first projection matmuls wait on ~1.5MB, not the full tensors.
        # cos/sin follow the first group (needed at the first rope evict); the
        # big wo tensor is emitted after P(b0) so it trickles in last.
        wq_sb = const.tile([128, CT, QH * D], BF)
        wk_sb = const.tile([128, CT, D], BF)
        wv_sb = const.tile([128, CT, D], BF)
        wqr = wq_d.rearrange("(ci p) n -> p ci n", p=128)
        wkr = wk_d.rearrange("(ci p) n -> p ci n", p=128)
        wvr = wv_d.rearrange("(ci p) n -> p ci n", p=128)
        cos_sb = const.tile([D, T], BF)
        sin_sb = const.tile([D, T], BF)
        GW = 8
        for g in range(0, CT, GW):
            s = slice(g, g + GW)
            nc.gpsimd.dma_start(wq_sb[:, s, :], wqr[:, s, :])
            nc.gpsimd.dma_start(wk_sb[:, s, :], wkr[:, s, :])
            nc.gpsimd.dma_start(wv_sb[:, s, :], wvr[:, s, :])
            if g == GW:
                nc.gpsimd.dma_start(cos_sb[:], cos_d)
                nc.gpsimd.dma_start(sin_sb[:], sin_d)
        alw_sb = const.tile([128, 4, 512], BF)
        nc.gpsimd.dma_start(alw_sb[:], alw_d)
        wo_sb = const.tile([128, QH, C], BF)
        onesbf_sb = const.tile([128, 128], BF)
        nc.gpsimd.memset(onesbf_sb[:], 1.0)
        bias_sb = const.tile([128, 1], F32)
        nc.gpsimd.memset(bias_sb[:], EXP_BIAS)

        def rope_sb(dst, src, cs):
            # dst = src * cos + swap_halves(src) * sin_rot   (all bf16 SBUF so
            # DVE runs in 2x/4x perf modes; src was evicted from PSUM by ACT)
            sw = work.tile([128, 512], BF, tag="sw", bufs=2, name="sw")
            nc.vector.tensor_copy(sw[0:64, :], src[64:128, :])
            nc.vector.tensor_copy(sw[64:128, :], src[0:64, :])
            nc.vector.tensor_mul(sw[:], sw[:], sin_sb[:, cs])
            cst = work.tile([128, 512], BF, tag="cst", bufs=2, name="cst")
            nc.vector.tensor_mul(cst[:], src[:], cos_sb[:, cs])
            nc.vector.tensor_add(dst, cst[:], sw[:])

        tr_pending = deque()

        def flush_tr(n):
            for _ in range(min(n, len(tr_pending))):
                tr_pending.popleft()()

        def proj_batch(pp, b):
            qT = act.tile([D, QH, T], BF, tag="qT", bufs=2, name="qT")
            kT = act.tile([D, T], BF, tag="kT", bufs=2, name="kT")
            vT = act.tile([D, T], BF, tag="vT", bufs=2, name="vT")
            vsb = act.tile([128, T // 128, D], BF, tag="v", bufs=2, name="vsb")
            for jc in range(NCH):
                # previous chunk's v transposes go on the ACT queue here, so
                # they sit between that chunk's bank-freeing copies and this
                # chunk's (the ACT queue is idle mid-chunk)
                flush_tr(4)
                pq = [pp.tile([128, 512], F32, tag=f"pq{h}", name=f"pq{h}")
                      for h in range(QH)]
                pk = pp.tile([128, 512], F32, tag="pk", bufs=2, name="pk")
                pv = pp.tile([128, 512], F32, tag="pv", bufs=2, name="pv")
                # q matmuls run SKEW c-tiles behind k/v so the previous
                # chunk's pq bank evictions are hidden behind ready work
                SKEW = 4
                xts = {}
                col0 = b * T + 512 * jc

                def q_mms(cq, h):
                    nc.tensor.matmul(
                        pq[h][:], wq_sb[:, cq, 128 * h:128 * (h + 1)],
                        xts[cq][:], start=cq == 0, stop=cq == CT - 1)

                for ci in range(CT):
                    xt = work.tile([128, 512], BF, tag="xt", bufs=12, name="xt")
                    xts[ci] = xt
                    nc.sync.dma_start(
                        xt[:], xT_d[128 * ci:128 * (ci + 1), col0:col0 + 512])
                    st, sp = ci == 0, ci == CT - 1
                    nc.tensor.matmul(pk[:], wk_sb[:, ci, :], xt[:],
                                     start=st, stop=sp)
                    nc.tensor.matmul(pv[:], wv_sb[:, ci, :], xt[:],
                                     start=st, stop=sp)
                    if ci >= SKEW:
                        for h in range(QH):
                            q_mms(ci - SKEW, h)
                        del xts[ci - SKEW]
                cs = slice(512 * jc, 512 * (jc + 1))
                # fast ACT copies free the PSUM banks so the next chunk's
                # matmuls never wait on the rope math (which runs SBUF-side
                # on DVE afterwards); kraw/vT copies overlap the q tail, and
                # the head-major tail staggers the pq stops so the qraw
                # copies pipeline against remaining tail matmuls
                kraw = work.tile([128, 512], BF, tag="kraw", bufs=2,
                                 name="kraw")
                nc.scalar.copy(kraw[:], pk[:])
                nc.scalar.copy(vT[:, cs], pv[:])
                qraws = []
                for h in range(QH):
                    for cq in range(CT - SKEW, CT):
                        q_mms(cq, h)
                    qraw = work.tile([128, 512], BF, tag="qraw", bufs=6,
                                     name="qraw")
                    nc.scalar.copy(qraw[:], pq[h][:])
                    qraws.append(qraw)
                rope_sb(kT[:, cs], kraw, cs)
                for h in range(QH):
                    rope_sb(qT[:, h, cs], qraws[h], cs)
                # v chunk -> [t, d] tiles via the DMA crossbar: one transpose
                # per chunk ([128,512] -> [128,4,128] maps vsb[p,k,d] =
                # vT[d,128k+p]), on the sync hwdge queue so its descriptor
                # cost never delays the ACT bank-freeing copies; deferred one
                # chunk so it cannot sit ahead of this chunk's copies
                tr_pending.append(
                    lambda jc=jc, cs=cs, vsb=vsb, vT=vT:
                        nc.sync.dma_start_transpose(
                            vsb[:, 4 * jc:4 * jc + 4, :], vT[:, cs]))
            return qT, kT, vsb

        with tc.tile_pool(name="pproj", bufs=1, space="PSUM") as pp:
            acts0 = proj_batch(pp, 0)
            # gate the wo load on a DVE op emitted here so the DMA cannot
            # start before P(b0) finishes -- keeps the HBM free for the xt
            # stream during the warmup chunks (wo is only read in attention)
            nc.vector.memset(wo_sb[:, 0, 0:8], 0.0)
            nc.gpsimd.dma_start(wo_sb[:], wo_d)
            acts1 = proj_batch(pp, 1)

        # ---- attention + output projection ----
        with tc.tile_pool(name="pattn", bufs=1, space="PSUM") as pa:
            wo_jobs = deque()

            def make_wo_job(b, j, tl, op, yts):
                # one job covers two adjacent 512-col output slices so the
                # store DMA gets 2KB lines (one [128,1024] bf16 transfer)
                def job():
                    ob = work.tile([128, 1024], BF, tag="ob", bufs=3,
                                   name="ob")
                    for half in range(2):
                        o = 2 * op + half
                        ops = pa.tile([128, 512], F32, tag="ops", bufs=2,
                                      name="ops")
                        for h in range(QH):
                            nc.tensor.matmul(
                                ops[:], yts[h][:, 128 * tl:128 * (tl + 1)],
                                wo_sb[:, h, 512 * o:512 * (o + 1)],
                                start=h == 0, stop=h == QH - 1)
                        # alternate the PSUM eviction between ACT and DVE so
                        # neither sidecar engine becomes the bottleneck
                        if half == 0:
                            nc.scalar.copy(ob[:, 0:512], ops[:])
                        else:
                            nc.vector.tensor_copy(ob[:, 512:1024], ops[:])
                    r0 = b * T + 512 * j + 128 * tl
                    nc.sync.dma_start(
                        out_d[r0:r0 + 128, 1024 * op:1024 * (op + 1)], ob[:])
                return job

            for b, (qT, kT, vsb) in ((0, acts0), (1, acts1)):
                for j in range(NCH):
                    yts = {}
                    for h in range(QH):
                        # sprinkle the last projection chunk's v transposes
                        # into the early attention heads (ACT queue slack)
                        flush_tr(1)
                        yps = pa.tile([128, 512], F32, tag="yps", bufs=1,
                                      name="yps")
                        K = 4 * j + 4
                        # pass 1: score matmuls stream; exp/mask/denominator
                        # trail on ACT/DVE. Diagonal tiles (o>=1) only touch
                        # their unmasked column range [128*o:512].
                        S = work.tile([128, 512], BF, tag="S", bufs=2,
                                      name="S")
                        pts = []
                        for k in range(K):
                            o = k - 4 * j
                            c0 = 128 * o if o > 0 else 0
                            sl = slice(c0, 512)
                            sps = pa.tile([128, 512], F32, tag="sps", bufs=4,
                                          name="sps")
                            nc.tensor.matmul(
                                sps[:, sl], kT[:, 128 * k:128 * (k + 1)],
                                qT[:, h, 512 * j + c0:512 * (j + 1)],
                                start=True, stop=True)
                            pt = work.tile([128, 512], BF, tag="pt", bufs=16,
                                           name="pt")
                            nc.scalar.activation(pt[:, sl], sps[:, sl], AFT.Exp,
                                                 bias=bias_sb[:], scale=SCALE)
                            if o >= 0:
                                nc.vector.tensor_mul(pt[:, sl], pt[:, sl],
                                                     alw_sb[:, o, sl])
                            if k == 0:
                                nc.vector.tensor_copy(S[:], pt[:])
                            else:
                                nc.vector.tensor_add(S[:, sl], S[:, sl],
                                                     pt[:, sl])
                            pts.append((pt, sl))
                            if wo_jobs:
                                wo_jobs.popleft()()
                        # pass 2: attn@v accumulation; k=0 always covers the
                        # full 512 columns so the start-matmul initializes the
                        # whole bank
                        for k, (pt, sl) in enumerate(pts):
                            nc.tensor.matmul(yps[:, sl], vsb[:, k, :],
                                             pt[:, sl],
                                             start=(k == 0), stop=(k == K - 1))
                            if wo_jobs:
                                wo_jobs.popleft()()
                        dns = pa.tile([128, 512], F32, tag="dns", bufs=1,
                                      name="dns")
                        nc.tensor.matmul(dns[:], onesbf_sb[:], S[:],
                                         start=True, stop=True)
                        rec = work.tile([128, 512], F32, tag="rec", bufs=1,
                                        name="rec")
                        nc.vector.reciprocal_approx_fast(rec[:], dns[:])
                        yt = work.tile([128, 512], BF, tag="yt", bufs=8,
                                       name="yt")
                        nc.vector.tensor_mul(yt[:], yps[:], rec[:])
                        yts[h] = yt
                    for tl in range(4):
                        for op in range(C // 1024):
                            wo_jobs.append(make_wo_job(b, j, tl, op, yts))
            while wo_jobs:
                wo_jobs.popleft()()

    nc.compile()
    return nc


def host_prep(inputs):
    x = np.asarray(inputs["x"], np.float32)
    mask = np.asarray(inputs["mask"], np.float32)
    wq = np.asarray(inputs["wq"], np.float32)
    wk = np.asarray(inputs["wk"], np.float32)
    wv = np.asarray(inputs["wv"], np.float32)
    wo = np.asarray(inputs["wo"], np.float32)

    xT = np.ascontiguousarray(x.reshape(B * T, C).T).astype(bf16)
    inv = 1.0 / (ROPE_BASE ** (np.arange(0, D, 2, dtype=np.float64) / D))
    freqs = np.arange(T, dtype=np.float64)[:, None] * inv[None, :] * B
    emb = np.concatenate([freqs, freqs], axis=-1)       # [T, D]
    cosT = np.cos(emb).T.astype(np.float32).astype(bf16)
    sinT = np.sin(emb).T.astype(np.float32)
    sinT[: D // 2] *= -1.0
    sinTr = sinT.astype(bf16)
    # allow[p, o, jj] = 1 - mask[jj, 128*o + p]  (from the actual mask input)
    allowA = np.ascontiguousarray(
        np.stack([(1.0 - mask[0:512, 128 * o:128 * (o + 1)]).T
                  for o in range(4)], axis=1)).astype(bf16)   # [128, 4, 512]

    common = dict(xT=xT, cosT=cosT, sinTr=sinTr, allowA=allowA)
    in_maps = []
    for c in range(NCORES):
        m = dict(common)
        m["wq"] = np.ascontiguousarray(wq[:, 512 * c:512 * (c + 1)]).astype(bf16)
        m["wk"] = np.ascontiguousarray(wk[:, 128 * c:128 * (c + 1)]).astype(bf16)
        m["wv"] = np.ascontiguousarray(wv[:, 128 * c:128 * (c + 1)]).astype(bf16)
        m["woA"] = np.ascontiguousarray(
            wo[512 * c:512 * (c + 1), :].reshape(QH, 128, C)
            .transpose(1, 0, 2)).astype(bf16)
        in_maps.append(m)
    return in_maps


def kernel(**inputs) -> np.ndarray:
    from concourse.bass_utils import run_bass_kernel_spmd

    in_maps = host_prep(inputs)
    nc = emit_program()
    trace = bool(os.environ.get("BASS_KERNEL_TRACE"))
    res = run_bass_kernel_spmd(nc, in_maps, core_ids=list(range(NCORES)),
                               trace=trace)
    if trace and res.exec_time_ns is not None:
        print(f"HW exec time: {res.exec_time_ns} ns")
        if res.instructions_and_trace is not None:
            print("trace:", res.instructions_and_trace[1])
    total = np.zeros((B * T, C), np.float32)
    for r in res.results:
        total += np.asarray(r["out"], dtype=np.float32)
    return total.reshape(B, T, C)


# revision 19
# speedup vs baseline: 1.3575x; 1.0080x over previous
"""Trainium2 Bass kernel for GQA attention (B=2, T=2048, C=4096, H=32, KV=8, D=128)
with RoPE and causal mask.

Sharding: tensor-parallel over heads across 8 cores. Each core owns 4 Q heads and
their shared KV head: projects q/k/v for those heads, runs causal attention, and
computes a partial output projection; the host sums the 8 partials (bf16 partials,
f32 accumulation on host).

All on-chip layouts are transposed ([feature, token]) so every matmul consumes
natural slices:
  qT/kT/vT = W^T @ x  via lhsT=W-tile [128c, cols], rhs=xT-tile [128c, 512t]
  sT[tk, tq] = kT-tile^T @ qT-chunk   (per 128-row key tile x 512-col query chunk;
               diagonal tiles stream only their unmasked column range)
  pT = exp(sT/sqrt(D) - 10) on ACT; strictly-causal-upper tiles skipped entirely
  S  = sum_k pT  accumulated on DVE (bf16) -> one ones-matmul per (b,h,j) gives
       the softmax denominator broadcast in PSUM (replaces a ones-matmul per tile)
  yT[d, tq] += v-tile^T @ pT          (v laid out [t, d] via DMA-crossbar transpose)
  out[tq, :] += yT_h^T @ wo_h         (accumulate 4 heads in PSUM, evict bf16, DMA)

Phase order is P(b0) P(b1) A(b0) A(b1) with double-buffered qT/kT/vsb so the PE
never sees a projection<->attention boundary stall. Output-projection matmul
"jobs" are popped from a queue inside the attention streams to keep the in-order
PE queue dense while ACT works through the exps.
"""

import os
from collections import deque
from contextlib import ExitStack

import numpy as np
import ml_dtypes

import concourse.bacc as bacc
import concourse.mybir as mybir
import concourse.tile as tile

BF = mybir.dt.bfloat16
F32 = mybir.dt.float32
AFT = mybir.ActivationFunctionType

NCORES = 8
B, T, C = 2, 2048, 4096
H, KV, D = 32, 8, 128
QH = H // NCORES          # 4 q-heads per core
CT = C // 128             # 32 contraction tiles
NCH = T // 512            # 4 query chunks per batch
SCALE = 1.0 / float(np.sqrt(D))
EXP_BIAS = -10.0
ROPE_BASE = 10000.0

bf16 = ml_dtypes.bfloat16


def emit_program():
    nc = bacc.Bacc("TRN2", target_bir_lowering=False, debug=False,
                   num_devices=NCORES)

    xT_d = nc.dram_tensor("xT", [C, B * T], BF, kind="ExternalInput").ap()
    wq_d = nc.dram_tensor("wq", [C, QH * D], BF, kind="ExternalInput").ap()
    wk_d = nc.dram_tensor("wk", [C, D], BF, kind="ExternalInput").ap()
    wv_d = nc.dram_tensor("wv", [C, D], BF, kind="ExternalInput").ap()
    wo_d = nc.dram_tensor("woA", [128, QH, C], BF, kind="ExternalInput").ap()
    cos_d = nc.dram_tensor("cosT", [D, T], BF, kind="ExternalInput").ap()
    sin_d = nc.dram_tensor("sinTr", [D, T], BF, kind="ExternalInput").ap()
    alw_d = nc.dram_tensor("allowA", [128, 4, 512], BF, kind="ExternalInput").ap()
    out_d = nc.dram_tensor("out", [B * T, C], BF, kind="ExternalOutput").ap()

    with tile.TileContext(nc) as tc, ExitStack() as ctx:
        const = ctx.enter_context(tc.tile_pool(name="const", bufs=1))
        act = ctx.enter_context(tc.tile_pool(name="act", bufs=1))
        work = ctx.enter_context(tc.tile_pool(name="work", bufs=1))

        # weights + tables on the gpsimd DMA queue so they never sit ahead of
        # the xt activation loads (sync queue); chunked in 8-c-tile groups so
        # the first projection matmuls wait on ~1.5MB, not the full tensors.
        # cos/sin follow the first group (needed at the first rope evict); the
        # big wo tensor is emitted after P(b0) so it trickles in last.
        wq_sb = const.tile([128, CT, QH * D], BF)
        wk_sb = const.tile([128, CT, D], BF)
        wv_sb = const.tile([128, CT, D], BF)
        wqr = wq_d.rearrange("(ci p) n -> p ci n", p=128)
        wkr = wk_d.rearrange("(ci p) n -> p ci n", p=128)
        wvr = wv_d.rearrange("(ci p) n -> p ci n", p=128)
        cos_sb = const.tile([D, T], BF)
        sin_sb = const.tile([D, T], BF)
        # finer groups early so the first matmuls start within ~2us, and the
        # rope tables only load once half the weights are in
        for g0, g1 in ((0, 2), (2, 4), (4, 8), (8, 16), (16, 24), (24, 32)):
            s = slice(g0, g1)
            nc.gpsimd.dma_start(wq_sb[:, s, :], wqr[:, s, :])
            nc.gpsimd.dma_start(wk_sb[:, s, :], wkr[:, s, :])
            nc.gpsimd.dma_start(wv_sb[:, s, :], wvr[:, s, :])
            if g1 == 24:
                nc.gpsimd.dma_start(cos_sb[:], cos_d)
                nc.gpsimd.dma_start(sin_sb[:], sin_d)
        alw_sb = const.tile([128, 4, 512], BF)
        wo_sb = const.tile([128, QH, C], BF)
        onesbf_sb = const.tile([128, 128], BF)
        nc.gpsimd.memset(onesbf_sb[:], 1.0)
        bias_sb = const.tile([128, 1], F32)
        nc.gpsimd.memset(bias_sb[:], EXP_BIAS)

        def rope_sb(dst, src, cs):
            # dst = src * cos + swap_halves(src) * sin_rot   (all bf16 SBUF so
            # DVE runs in 2x/4x perf modes; src was evicted from PSUM by ACT)
            sw = work.tile([128, 512], BF, tag="sw", bufs=2, name="sw")
            nc.vector.tensor_copy(sw[0:64, :], src[64:128, :])
            nc.vector.tensor_copy(sw[64:128, :], src[0:64, :])
            nc.vector.tensor_mul(sw[:], sw[:], sin_sb[:, cs])
            cst = work.tile([128, 512], BF, tag="cst", bufs=2, name="cst")
            nc.vector.tensor_mul(cst[:], src[:], cos_sb[:, cs])
            nc.vector.tensor_add(dst, cst[:], sw[:])

        tr_pending = deque()

        def flush_tr(n):
            for _ in range(min(n, len(tr_pending))):
                tr_pending.popleft()()

        def proj_batch(pp, b):
            qT = act.tile([D, QH, T], BF, tag="qT", bufs=2, name="qT")
            kT = act.tile([D, T], BF, tag="kT", bufs=2, name="kT")
            vT = act.tile([D, T], BF, tag="vT", bufs=2, name="vT")
            vsb = act.tile([128, T // 128, D], BF, tag="v", bufs=2, name="vsb")
            for jc in range(NCH):
                # previous chunk's v transposes go on the ACT queue here, so
                # they sit between that chunk's bank-freeing copies and this
                # chunk's (the ACT queue is idle mid-chunk)
                flush_tr(4)
                pq = [pp.tile([128, 512], F32, tag=f"pq{h}", name=f"pq{h}")
                      for h in range(QH)]
                pk = pp.tile([128, 512], F32, tag="pk", bufs=2, name="pk")
                pv = pp.tile([128, 512], F32, tag="pv", bufs=2, name="pv")
                # q matmuls run SKEW c-tiles behind k/v so the previous
                # chunk's pq bank evictions are hidden behind ready work
                SKEW = 4
                xts = {}
                col0 = b * T + 512 * jc

                def q_mms(cq, h):
                    nc.tensor.matmul(
                        pq[h][:], wq_sb[:, cq, 128 * h:128 * (h + 1)],
                        xts[cq][:], start=cq == 0, stop=cq == CT - 1)

                for ci in range(CT):
                    xt = work.tile([128, 512], BF, tag="xt", bufs=13, name="xt")
                    xts[ci] = xt
                    nc.sync.dma_start(
                        xt[:], xT_d[128 * ci:128 * (ci + 1), col0:col0 + 512])
                    st, sp = ci == 0, ci == CT - 1
                    nc.tensor.matmul(pk[:], wk_sb[:, ci, :], xt[:],
                                     start=st, stop=sp)
                    nc.tensor.matmul(pv[:], wv_sb[:, ci, :], xt[:],
                                     start=st, stop=sp)
                    if ci >= SKEW:
                        for h in range(QH):
                            q_mms(ci - SKEW, h)
                        del xts[ci - SKEW]
                cs = slice(512 * jc, 512 * (jc + 1))
                # fast ACT copies free the PSUM banks so the next chunk's
                # matmuls never wait on the rope math (which runs SBUF-side
                # on DVE afterwards); kraw/vT copies overlap the q tail, and
                # the head-major tail staggers the pq stops so the qraw
                # copies pipeline against remaining tail matmuls
                kraw = work.tile([128, 512], BF, tag="kraw", bufs=2,
                                 name="kraw")
                nc.scalar.copy(kraw[:], pk[:])
                nc.scalar.copy(vT[:, cs], pv[:])
                qraws = []
                for h in range(QH):
                    for cq in range(CT - SKEW, CT):
                        q_mms(cq, h)
                    qraw = work.tile([128, 512], BF, tag="qraw", bufs=5,
                                     name="qraw")
                    nc.scalar.copy(qraw[:], pq[h][:])
                    qraws.append(qraw)
                rope_sb(kT[:, cs], kraw, cs)
                for h in range(QH):
                    rope_sb(qT[:, h, cs], qraws[h], cs)
                # v chunk -> [t, d] tiles via the DMA crossbar: one transpose
                # per chunk ([128,512] -> [128,4,128] maps vsb[p,k,d] =
                # vT[d,128k+p]), on the sync hwdge queue so its descriptor
                # cost never delays the ACT bank-freeing copies; deferred one
                # chunk so it cannot sit ahead of this chunk's copies
                tr_pending.append(
                    lambda jc=jc, cs=cs, vsb=vsb, vT=vT:
                        nc.sync.dma_start_transpose(
                            vsb[:, 4 * jc:4 * jc + 4, :], vT[:, cs]))
            return qT, kT, vsb

        with tc.tile_pool(name="pproj", bufs=1, space="PSUM") as pp:
            acts0 = proj_batch(pp, 0)
            # gate the wo/alw loads on a DVE op emitted here so the DMA
            # cannot start before P(b0) finishes -- keeps the HBM free for
            # the xt stream during the warmup chunks (both are only read in
            # the attention phase)
            nc.vector.memset(wo_sb[:, 0, 0:8], 0.0)
            nc.vector.memset(alw_sb[:, 0, 0:8], 0.0)
            nc.gpsimd.dma_start(alw_sb[:], alw_d)
            nc.gpsimd.dma_start(wo_sb[:], wo_d)
            acts1 = proj_batch(pp, 1)

        # ---- attention + output projection ----
        with tc.tile_pool(name="pattn", bufs=1, space="PSUM") as pa:
            wo_jobs = deque()

            def make_wo_job(b, j, tl, op, yts):
                # one job covers two adjacent 512-col output slices so the
                # store DMA gets 2KB lines (one [128,1024] bf16 transfer)
                def job():
                    ob = work.tile([128, 1024], BF, tag="ob", bufs=3,
                                   name="ob")
                    for half in range(2):
                        o = 2 * op + half
                        ops = pa.tile([128, 512], F32, tag="ops", bufs=2,
                                      name="ops")
                        for h in range(QH):
                            nc.tensor.matmul(
                                ops[:], yts[h][:, 128 * tl:128 * (tl + 1)],
                                wo_sb[:, h, 512 * o:512 * (o + 1)],
                                start=h == 0, stop=h == QH - 1)
                        # alternate the PSUM eviction between ACT and DVE so
                        # neither sidecar engine becomes the bottleneck
                        if half == 0:
                            nc.scalar.copy(ob[:, 0:512], ops[:])
                        else:
                            nc.vector.tensor_copy(ob[:, 512:1024], ops[:])
                    r0 = b * T + 512 * j + 128 * tl
                    nc.sync.dma_start(
                        out_d[r0:r0 + 128, 1024 * op:1024 * (op + 1)], ob[:])
                return job

            for b, (qT, kT, vsb) in ((0, acts0), (1, acts1)):
                for j in range(NCH):
                    yts = {}
                    for h in range(QH):
                        # sprinkle the last projection chunk's v transposes
                        # into the early attention heads (ACT queue slack)
                        flush_tr(1)
                        yps = pa.tile([128, 512], F32, tag="yps", bufs=1,
                                      name="yps")
                        K = 4 * j + 4
                        # pass 1: score matmuls stream; exp/mask/denominator
                        # trail on ACT/DVE. Diagonal tiles (o>=1) only touch
                        # their unmasked column range [128*o:512].
                        S = work.tile([128, 512], BF, tag="S", bufs=2,
                                      name="S")
                        pts = []
                        for k in range(K):
                            o = k - 4 * j
                            c0 = 128 * o if o > 0 else 0
                            sl = slice(c0, 512)
                            sps = pa.tile([128, 512], F32, tag="sps", bufs=4,
                                          name="sps")
                            nc.tensor.matmul(
                                sps[:, sl], kT[:, 128 * k:128 * (k + 1)],
                                qT[:, h, 512 * j + c0:512 * (j + 1)],
                                start=True, stop=True)
                            pt = work.tile([128, 512], BF, tag="pt", bufs=16,
                                           name="pt")
                            nc.scalar.activation(pt[:, sl], sps[:, sl], AFT.Exp,
                                                 bias=bias_sb[:], scale=SCALE)
                            if o >= 0:
                                nc.vector.tensor_mul(pt[:, sl], pt[:, sl],
                                                     alw_sb[:, o, sl])
                            if k == 0:
                                nc.vector.tensor_copy(S[:], pt[:])
                            else:
                                nc.vector.tensor_add(S[:, sl], S[:, sl],
                                                     pt[:, sl])
                            pts.append((pt, sl))
                            if wo_jobs:
                                wo_jobs.popleft()()
                        # pass 2: attn@v accumulation; k=0 always covers the
                        # full 512 columns so the start-matmul initializes the
                        # whole bank
                        for k, (pt, sl) in enumerate(pts):
                            nc.tensor.matmul(yps[:, sl], vsb[:, k, :],
                                             pt[:, sl],
                                             start=(k == 0), stop=(k == K - 1))
                            if wo_jobs:
                                wo_jobs.popleft()()
                        dns = pa.tile([128, 512], F32, tag="dns", bufs=1,
                                      name="dns")
                        nc.tensor.matmul(dns[:], onesbf_sb[:], S[:],
                                         start=True, stop=True)
                        rec = work.tile([128, 512], F32, tag="rec", bufs=1,
                                        name="rec")
                        nc.vector.reciprocal_approx_fast(rec[:], dns[:])
                        yt = work.tile([128, 512], BF, tag="yt", bufs=8,
                                       name="yt")
                        nc.vector.tensor_mul(yt[:], yps[:], rec[:])
                        yts[h] = yt
                    for tl in range(4):
                        for op in range(C // 1024):
                            wo_jobs.append(make_wo_job(b, j, tl, op, yts))
            while wo_jobs:
                wo_jobs.popleft()()

    nc.compile()
    return nc


def host_prep(inputs):
    x = np.asarray(inputs["x"], np.float32)
    mask = np.asarray(inputs["mask"], np.float32)
    wq = np.asarray(inputs["wq"], np.float32)
    wk = np.asarray(inputs["wk"], np.float32)
    wv = np.asarray(inputs["wv"], np.float32)
    wo = np.asarray(inputs["wo"], np.float32)

    xT = np.ascontiguousarray(x.reshape(B * T, C).T).astype(bf16)
    inv = 1.0 / (ROPE_BASE ** (np.arange(0, D, 2, dtype=np.float64) / D))
    freqs = np.arange(T, dtype=np.float64)[:, None] * inv[None, :] * B
    emb = np.concatenate([freqs, freqs], axis=-1)       # [T, D]
    cosT = np.cos(emb).T.astype(np.float32).astype(bf16)
    sinT = np.sin(emb).T.astype(np.float32)
    sinT[: D // 2] *= -1.0
    sinTr = sinT.astype(bf16)
    # allow[p, o, jj] = 1 - mask[jj, 128*o + p]  (from the actual mask input)
    allowA = np.ascontiguousarray(
        np.stack([(1.0 - mask[0:512, 128 * o:128 * (o + 1)]).T
                  for o in range(4)], axis=1)).astype(bf16)   # [128, 4, 512]

    common = dict(xT=xT, cosT=cosT, sinTr=sinTr, allowA=allowA)
    in_maps = []
    for c in range(NCORES):
        m = dict(common)
        m["wq"] = np.ascontiguousarray(wq[:, 512 * c:512 * (c + 1)]).astype(bf16)
        m["wk"] = np.ascontiguousarray(wk[:, 128 * c:128 * (c + 1)]).astype(bf16)
        m["wv"] = np.ascontiguousarray(wv[:, 128 * c:128 * (c + 1)]).astype(bf16)
        m["woA"] = np.ascontiguousarray(
            wo[512 * c:512 * (c + 1), :].reshape(QH, 128, C)
            .transpose(1, 0, 2)).astype(bf16)
        in_maps.append(m)
    return in_maps


def kernel(**inputs) -> np.ndarray:
    from concourse.bass_utils import run_bass_kernel_spmd

    in_maps = host_prep(inputs)
    nc = emit_program()
    trace = bool(os.environ.get("BASS_KERNEL_TRACE"))
    res = run_bass_kernel_spmd(nc, in_maps, core_ids=list(range(NCORES)),
                               trace=trace)
    if trace and res.exec_time_ns is not None:
        print(f"HW exec time: {res.exec_time_ns} ns")
        if res.instructions_and_trace is not None:
            print("trace:", res.instructions_and_trace[1])
    total = np.zeros((B * T, C), np.float32)
    for r in res.results:
        total += np.asarray(r["out"], dtype=np.float32)
    return total.reshape(B, T, C)
